# revision 13
# baseline (speedup 1.0000x reference)
"""Trainium2 Bass kernel for CrossScaleInteractionNormalization.

Contract: kernel(**inputs) takes the FULL unsharded inputs (numpy arrays,
keyed as in setup_inputs) and returns the full (out, attn_weights) pair.

Strategy
--------
Data-parallel over batch: core b handles batch b (B == n_cores == 8).
Attention is per-batch, so no cross-core communication is needed.

Math restructure (exact, verified against the reference):
- Only (p, q!=p) interaction matrices are ever selected, and the
  selection is a per-token one-hot over 3 scale types.  Premask x on the
  host into xm[p] = x * (scale_types == p) and compute
  inter = sum_p xm[p] @ concat(M[p, o0(p)], M[p, o1(p)])   (1024 cols)
  The zero-pad to 1536 cols means only Wq/Wk/Wv rows [:1024] matter.
- attended = ctx @ Wo + bo feeds only the MLP, so fold WoW1 = Wo @ W1,
  b1' = bo @ W1 + b1.
- softmax without max-subtraction (scores are O(5); fp32 exp is safe);
  normalization by 1/(8*rowsum) is folded into the prob matrix so the
  8-head mean for attn_weights is a plain accumulation, and the ctx
  eviction multiplies by 8 to undo it.

All matmuls run in float32r (full fp32 storage; ~1e-4 matmul error at
bf16 speed for free-dim >= 256).  The softmax prob matrix is cast to
bf16 so the q->k transpose can use the DMA xbar transpose and the ctx
matmul runs in bf16.  Activations stay feature-major ([feat, token]) so
each matmul's output feeds the next as the moving operand directly.
"""

import numpy as np

NS, D, E, NH = 3, 512, 1536, 8
HD = E // NH          # 192
B, S = 8, 1024
P = 128
EPS = 1e-5
NCORES = 8
OTH = ((1, 2), (0, 2), (0, 1))

_CACHE = {}


def _build():
    import concourse.mybir as mybir
    import concourse.tile as tile
    from concourse import bacc

    f32 = mybir.dt.float32
    f32r = mybir.dt.float32r

    nc = bacc.Bacc("TRN2", target_bir_lowering=False, debug=False,
                   num_devices=NCORES)

    d = {}
    d["xm"] = nc.dram_tensor("xm", [NS, D, S], f32r, kind="ExternalInput")
    d["mcat"] = nc.dram_tensor("mcat", [NS, D, 2 * D], f32r,
                               kind="ExternalInput")
    d["wq"] = nc.dram_tensor("wq", [2 * D, E], f32r, kind="ExternalInput")
    d["wk"] = nc.dram_tensor("wk", [2 * D, E], f32r, kind="ExternalInput")
    d["wv"] = nc.dram_tensor("wv", [2 * D, E], f32r, kind="ExternalInput")
    d["wow1"] = nc.dram_tensor("wow1", [E, D], f32r, kind="ExternalInput")
    d["w2"] = nc.dram_tensor("w2", [D, D], f32r, kind="ExternalInput")
    d["bqh"] = nc.dram_tensor("bqh", [16, P], f32, kind="ExternalInput")
    d["bkh"] = nc.dram_tensor("bkh", [16, P], f32, kind="ExternalInput")
    d["bvh"] = nc.dram_tensor("bvh", [16, P], f32, kind="ExternalInput")
    d["b1p"] = nc.dram_tensor("b1p", [4, P], f32, kind="ExternalInput")
    d["b2r"] = nc.dram_tensor("b2r", [1, D], f32, kind="ExternalInput")
    d["gbc"] = nc.dram_tensor("gbc", [P, D], f32, kind="ExternalInput")
    d["bbc"] = nc.dram_tensor("bbc", [P, D], f32, kind="ExternalInput")
    d["ctxs"] = nc.dram_tensor("ctxs", [12, P, S], f32r)     # scratch
    d["out"] = nc.dram_tensor("out", [S, D], f32, kind="ExternalOutput")
    d["aw"] = nc.dram_tensor("aw", [S, S], f32, kind="ExternalOutput")

    with tile.TileContext(nc) as tc:
        _emit(nc, tc, d)
    nc.compile()
    return nc


def _emit(nc, tc, d):
    import concourse.mybir as mybir

    f32 = mybir.dt.float32
    f32r = mybir.dt.float32r
    bf16 = mybir.dt.bfloat16
    AF = mybir.ActivationFunctionType
    ALU = mybir.AluOpType

    from contextlib import ExitStack
    es = ExitStack()
    with es:
        pc = es.enter_context(tc.tile_pool(name="consts", bufs=1))
        pms = es.enter_context(tc.tile_pool(name="pms", bufs=4))
        psA = es.enter_context(tc.tile_pool(name="psA", bufs=2, space="PSUM"))
        psB = es.enter_context(tc.tile_pool(name="psB", bufs=2, space="PSUM"))

        # ---------- consts ----------
        ones_t = pc.tile([1, P], f32, tag="ones")
        nc.any.memset(ones_t[:], 1.0)
        b2_t = pc.tile([1, D], f32, tag="b2")
        nc.sync.dma_start(b2_t[:], d["b2r"][:])
        gbc_t = pc.tile([P, D], f32, tag="gbc")
        nc.sync.dma_start(gbc_t[:], d["gbc"][:])
        bbc_t = pc.tile([P, D], f32, tag="bbc")
        nc.sync.dma_start(bbc_t[:], d["bbc"][:])
        bqh_t = pc.tile([P, 16], f32, tag="bqh")
        nc.sync.dma_start(bqh_t[:], d["bqh"].rearrange("c p -> p c"))
        bkh_t = pc.tile([P, 16], f32, tag="bkh")
        nc.sync.dma_start(bkh_t[:], d["bkh"].rearrange("c p -> p c"))
        bvh_t = pc.tile([P, 16], f32, tag="bvh")
        nc.sync.dma_start(bvh_t[:], d["bvh"].rearrange("c p -> p c"))
        b1p_t = pc.tile([P, 4], f32, tag="b1p")
        nc.sync.dma_start(b1p_t[:], d["b1p"].rearrange("c p -> p c"))

        with tc.tile_pool(name="pint", bufs=1) as pint:
            intT = pint.tile([P, 8, S], f32r, tag="intT")      # inter.T

            # ---- phase 1a: interT = (sum_p xm_p.T @ Mcat_p).T ----
            with tc.tile_pool(name="pxm", bufs=1) as pxm, \
                 tc.tile_pool(name="pmcat", bufs=1) as pmcat:
                xm_t = pxm.tile([P, 12, S], f32r, tag="xm")
                mc_t = pmcat.tile([P, 12, 2 * D], f32r, tag="mc")
                for p in range(NS):
                    for c in range(4):
                        nc.sync.dma_start(xm_t[:, p * 4 + c, :],
                                          d["xm"][p, c * P:(c + 1) * P, :])
                        nc.sync.dma_start(mc_t[:, p * 4 + c, :],
                                          d["mcat"][p, c * P:(c + 1) * P, :])
                for j in range(8):
                    for n in range(2):
                        ps = psB.tile([P, 512], f32, tag="pj")
                        for idx in range(12):
                            nc.tensor.matmul(
                                ps[:],
                                mc_t[:, idx, j * P:(j + 1) * P],
                                xm_t[:, idx, n * 512:(n + 1) * 512],
                                start=(idx == 0), stop=(idx == 11))
                        nc.scalar.copy(intT[:, j, n * 512:(n + 1) * 512],
                                       ps[:])

            with tc.tile_pool(name="pv", bufs=1) as pv:
                vt = pv.tile([P, 8, E], bf16, tag="vt")    # v token-major

                # ---- phase 1b: v ----
                with tc.tile_pool(name="pwv", bufs=1) as pwv:
                    wv_t = pwv.tile([P, 8, E], f32r, tag="wv")
                    for c in range(8):
                        nc.sync.dma_start(wv_t[:, c, :],
                                          d["wv"][c * P:(c + 1) * P, :])
                    for tt in range(8):
                        for nn in range(3):
                            ps = psB.tile([P, 512], f32, tag="pj")
                            for c in range(8):
                                nc.tensor.matmul(
                                    ps[:],
                                    intT[:, c, tt * P:(tt + 1) * P],
                                    wv_t[:, c, nn * 512:(nn + 1) * 512],
                                    start=(c == 0), stop=(c == 7))
                            nc.vector.tensor_copy(
                                vt[:, tt, nn * 512:(nn + 1) * 512], ps[:])

                # ---- phase 2: per-head attention ----
                # scores are computed TRANSPOSED (scT[k_tok, q_tok]), so the
                # prob matrix comes out k-major and feeds the ctx matmul
                # directly (no transposes).  Row sums (softmax denominators,
                # per q) come from a PE ones-reduction over eT; their
                # reciprocal rows are broadcast across partitions by a K=1
                # matmul.  attn_weights accumulates transposed and is
                # un-transposed on the host.
                with tc.tile_pool(name="pmw", bufs=1) as pmw, \
                     tc.tile_pool(name="pqk", bufs=1) as pqk, \
                     tc.tile_pool(name="pet", bufs=2) as pet, \
                     tc.tile_pool(name="prb", bufs=2) as prb, \
                     tc.tile_pool(name="ptmp", bufs=2) as ptmp, \
                     tc.tile_pool(name="pwsm", bufs=1) as pwsm, \
                     tc.tile_pool(name="pcx", bufs=4) as pcx, \
                     tc.tile_pool(name="psD", bufs=1, space="PSUM") as psD:
                    onesc = pc.tile([P, 1], bf16, tag="onesc")
                    nc.any.memset(onesc[:], 1.0)
                    mwT = pmw.tile([P, 8, S], f32, tag="mwT")
                    for h in range(8):
                        wqh = pwsm.tile([P, 8, HD], f32r, tag="wqh")
                        wkh = pwsm.tile([P, 8, HD], f32r, tag="wkh")
                        for c in range(8):
                            nc.sync.dma_start(
                                wqh[:, c, :],
                                d["wq"][c * P:(c + 1) * P,
                                        h * HD:(h + 1) * HD])
                            nc.sync.dma_start(
                                wkh[:, c, :],
                                d["wk"][c * P:(c + 1) * P,
                                        h * HD:(h + 1) * HD])

                        qh_a = pqk.tile([P, S], f32r, tag="qh_a")
                        qh_b = pqk.tile([64, S], f32r, tag="qh_b")
                        kh_a = pqk.tile([P, S], f32r, tag="kh_a")
                        kh_b = pqk.tile([64, S], f32r, tag="kh_b")
                        for (dsta, dstb, wt, bias) in (
                                (qh_a, qh_b, wqh, bqh_t),
                                (kh_a, kh_b, wkh, bkh_t)):
                            for a in range(2):
                                rows = P if a == 0 else 64
                                dst = dsta if a == 0 else dstb
                                for n in range(2):
                                    ps = psB.tile([P, 512], f32, tag="pj")
                                    for c in range(8):
                                        nc.tensor.matmul(
                                            ps[:rows, :],
                                            wt[:, c, a * P:a * P + rows],
                                            intT[:, c,
                                                 n * 512:(n + 1) * 512],
                                            start=(c == 0), stop=(c == 7))
                                    nc.vector.tensor_scalar_add(
                                        dst[:rows, n * 512:(n + 1) * 512],
                                        ps[:rows, :],
                                        bias[:rows,
                                             2 * h + a:2 * h + a + 1])

                        # eT[k, q] = exp(scT), unnormalized, bf16
                        eT = pet.tile([P, 8, S], bf16, tag="eT")
                        dn0 = psD.tile([1, 512], f32, tag="dn0")
                        dn1 = psD.tile([1, 512], f32, tag="dn1")
                        for kt in range(8):
                            sc = psA.tile([P, S], f32, tag="sc")
                            for n in range(2):
                                nc.tensor.matmul(
                                    sc[:, n * 512:(n + 1) * 512],
                                    kh_a[:, kt * P:(kt + 1) * P],
                                    qh_a[:, n * 512:(n + 1) * 512],
                                    start=True, stop=False)
                                nc.tensor.matmul(
                                    sc[:, n * 512:(n + 1) * 512],
                                    kh_b[:, kt * P:(kt + 1) * P],
                                    qh_b[:, n * 512:(n + 1) * 512],
                                    start=False, stop=True)
                            nc.scalar.activation(eT[:, kt, :], sc[:], AF.Exp)
                            # denominators: den[q] += sum_k eT[k, q] (PE)
                            for n, dn in ((0, dn0), (1, dn1)):
                                nc.tensor.matmul(
                                    dn[:], onesc[:],
                                    eT[:, kt, n * 512:(n + 1) * 512],
                                    start=(kt == 0), stop=(kt == 7))
                        # r_row = 1/(8*den);  rbc = broadcast over partitions
                        rrow = prb.tile([1, S], f32, tag="rrow")
                        nc.vector.tensor_scalar_mul(rrow[:, :512], dn0[:], 8.0)
                        nc.vector.tensor_scalar_mul(rrow[:, 512:], dn1[:], 8.0)
                        nc.vector.reciprocal(rrow[:], rrow[:])
                        rbc = prb.tile([P, S], f32, tag="rbc")
                        for n in range(2):
                            psr = psB.tile([P, 512], f32, tag="pj")
                            nc.tensor.matmul(
                                psr[:], ones_t[:],
                                rrow[:, n * 512:(n + 1) * 512],
                                start=True, stop=True)
                            nc.vector.tensor_copy(
                                rbc[:, n * 512:(n + 1) * 512], psr[:])
                        # mwT[k, q] += eT * rbc   (attn/8, transposed)
                        for kt in range(8):
                            if h == 0:
                                nc.vector.tensor_tensor(
                                    mwT[:, kt, :], eT[:, kt, :], rbc[:],
                                    ALU.mult)
                            else:
                                tmp = ptmp.tile([P, S], f32, tag="tmp")
                                nc.vector.tensor_tensor(
                                    tmp[:], eT[:, kt, :], rbc[:], ALU.mult)
                                nc.gpsimd.tensor_tensor(
                                    mwT[:, kt, :], mwT[:, kt, :], tmp[:],
                                    ALU.add)
                            if h == 7:
                                nc.sync.dma_start(
                                    d["aw"][kt * P:(kt + 1) * P, :],
                                    mwT[:, kt, :])

                        # ctx.T[dv, q] = (8 * sum_k v[k,dv] eT[k,q]) * rbc + bv
                        for a in range(2):
                            rows = P if a == 0 else 64
                            for n in range(2):
                                ps = psB.tile([P, 512], f32, tag="pj")
                                for c in range(8):
                                    nc.tensor.matmul(
                                        ps[:rows, :],
                                        vt[:, c, h * HD + a * P:
                                           h * HD + a * P + rows],
                                        eT[:, c, n * 512:(n + 1) * 512],
                                        start=(c == 0), stop=(c == 7))
                                cx = pcx.tile([P, 512], f32r, tag="cx")
                                if a == 0:
                                    bias = bvh_t[:, 2 * h:2 * h + 1]
                                    row_c, row_0 = h, 0
                                else:
                                    row_0 = 64 * (h % 2)
                                    bias = bvh_t[row_0:row_0 + 64,
                                                 2 * h + 1:2 * h + 2]
                                    row_c = 8 + h // 2
                                nc.vector.scalar_tensor_tensor(
                                    cx[:rows, :], ps[:rows, :], 8.0,
                                    rbc[:rows, n * 512:(n + 1) * 512],
                                    ALU.mult, ALU.mult)
                                nc.vector.tensor_scalar_add(
                                    cx[:rows, :], cx[:rows, :], bias)
                                nc.sync.dma_start(
                                    d["ctxs"][row_c,
                                              row_0:row_0 + rows,
                                              n * 512:(n + 1) * 512],
                                    cx[:rows, :])

        # ---- phase 3: h1.T = silu(WoW1.T @ ctx.T + b1') ----
        with tc.tile_pool(name="ph1", bufs=1) as ph1:
            h1T = ph1.tile([P, 4, S], f32r, tag="h1T")
            with tc.tile_pool(name="pwo", bufs=1) as pwo, \
                 tc.tile_pool(name="pcs", bufs=4) as pcs, \
                 tc.tile_pool(name="psE", bufs=2, space="PSUM") as psE:
                wo_t = pwo.tile([P, 12, D], f32r, tag="wo")
                for c in range(12):
                    nc.sync.dma_start(wo_t[:, c, :],
                                      d["wow1"][c * P:(c + 1) * P, :])
                for n in range(2):
                    pss = []
                    for _m in range(4):
                        pool_m = psB if _m < 2 else psE
                        ps_m = pool_m.tile([P, 512], f32, tag="pj")
                        pss.append(ps_m)
                    for c in range(12):
                        cxs = pcs.tile([P, 512], f32r, tag="cxs")
                        nc.sync.dma_start(
                            cxs[:], d["ctxs"][c, :, n * 512:(n + 1) * 512])
                        for m in range(4):
                            nc.tensor.matmul(
                                pss[m][:], wo_t[:, c, m * P:(m + 1) * P],
                                cxs[:], start=(c == 0), stop=(c == 11))
                    for m in range(4):
                        nc.scalar.activation(
                            h1T[:, m, n * 512:(n + 1) * 512], pss[m][:],
                            AF.Silu, bias=b1p_t[:, m:m + 1])

            # ---- phase 4: h2 = h1 @ W2 + b2 (token-major), LayerNorm ----
            with tc.tile_pool(name="pw2", bufs=1) as pw2, \
                 tc.tile_pool(name="pln", bufs=2) as pln:
                w2_t = pw2.tile([P, 4, D], f32r, tag="w2")
                for c in range(4):
                    nc.sync.dma_start(w2_t[:, c, :],
                                      d["w2"][c * P:(c + 1) * P, :])
                for tt in range(8):
                    ps = psB.tile([P, 512], f32, tag="pj")
                    for c in range(4):
                        nc.tensor.matmul(
                            ps[:], h1T[:, c, tt * P:(tt + 1) * P],
                            w2_t[:, c, :], start=(c == 0), stop=False)
                    nc.tensor.matmul(ps[:], ones_t[:], b2_t[:],
                                     start=False, stop=True)
                    h2s = pln.tile([P, D], f32, tag="h2s")
                    ssum = pms.tile([P, 1], f32, tag="ssum")
                    nc.scalar.activation(h2s[:], ps[:], AF.Copy,
                                         accum_out=ssum[:])
                    sq = pln.tile([P, D], f32, tag="sq")
                    s2 = pms.tile([P, 1], f32, tag="s2")
                    nc.scalar.activation(sq[:], ps[:], AF.Square,
                                         accum_out=s2[:])
                    mu = pms.tile([P, 1], f32, tag="mu")
                    nc.vector.tensor_scalar_mul(mu[:], ssum[:], 1.0 / D)
                    ex2 = pms.tile([P, 1], f32, tag="ex2")
                    nc.vector.tensor_scalar_mul(ex2[:], s2[:], 1.0 / D)
                    var = pms.tile([P, 1], f32, tag="var")
                    nc.vector.tensor_tensor(var[:], mu[:], mu[:], ALU.mult)
                    nc.vector.tensor_tensor(var[:], ex2[:], var[:],
                                            ALU.subtract)
                    nc.vector.tensor_scalar_add(var[:], var[:], EPS)
                    sd = pms.tile([P, 1], f32, tag="sd")
                    nc.scalar.activation(sd[:], var[:], AF.Sqrt)
                    rstd = pms.tile([P, 1], f32, tag="rstd")
                    nc.vector.reciprocal(rstd[:], sd[:])
                    nmu = pms.tile([P, 1], f32, tag="nmu")
                    nc.vector.tensor_tensor(nmu[:], mu[:], rstd[:], ALU.mult)
                    nc.vector.tensor_scalar_mul(nmu[:], nmu[:], -1.0)
                    nrm = pln.tile([P, D], f32, tag="nrm")
                    nc.vector.tensor_scalar(nrm[:], h2s[:], rstd[:], nmu[:],
                                            ALU.mult, ALU.add)
                    nc.vector.tensor_tensor(nrm[:], nrm[:], gbc_t[:],
                                            ALU.mult)
                    nc.vector.tensor_tensor(nrm[:], nrm[:], bbc_t[:],
                                            ALU.add)
                    nc.sync.dma_start(d["out"][tt * P:(tt + 1) * P, :],
                                      nrm[:])


def _host_prep(inputs):
    x = np.asarray(inputs["x"], np.float32)
    st = np.asarray(inputs["scale_types"])
    M = np.asarray(inputs["M"], np.float32)
    Wq = np.asarray(inputs["Wq"], np.float32)[:2 * D]
    Wk = np.asarray(inputs["Wk"], np.float32)[:2 * D]
    Wv = np.asarray(inputs["Wv"], np.float32)[:2 * D]
    bq = np.asarray(inputs["bq"], np.float32)
    bk = np.asarray(inputs["bk"], np.float32)
    bv = np.asarray(inputs["bv"], np.float32)
    Wo = np.asarray(inputs["Wo"], np.float32)
    bo = np.asarray(inputs["bo"], np.float32)
    W1 = np.asarray(inputs["W1"], np.float32)
    b1 = np.asarray(inputs["b1"], np.float32)
    W2 = np.asarray(inputs["W2"], np.float32)
    b2 = np.asarray(inputs["b2"], np.float32)
    ln_g = np.asarray(inputs["ln_g"], np.float32)
    ln_b = np.asarray(inputs["ln_b"], np.float32)

    sq = 1.0 / np.sqrt(np.float32(HD))
    mcat = np.stack([np.concatenate([M[p, OTH[p][0]], M[p, OTH[p][1]]],
                                    axis=1) for p in range(NS)])
    wq_s = np.ascontiguousarray(Wq * sq)
    bq_s = bq * sq
    wow1 = Wo @ W1
    b1p = bo @ W1 + b1
    permA = np.concatenate([np.arange(h * HD, h * HD + P) for h in range(NH)])
    permB = np.concatenate([np.arange(h * HD + P, (h + 1) * HD)
                            for h in range(NH)])
    wow1p = np.ascontiguousarray(wow1[np.concatenate([permA, permB])])

    def head_bias(b):
        out = np.zeros((16, P), np.float32)
        for h in range(NH):
            out[2 * h, :] = b[h * HD:h * HD + P]
            seg = b[h * HD + P:(h + 1) * HD]
            out[2 * h + 1, :64] = seg
            out[2 * h + 1, 64:] = seg
        return out

    common = {
        "mcat": mcat, "wq": wq_s, "wk": np.ascontiguousarray(Wk),
        "wv": np.ascontiguousarray(Wv), "wow1": wow1p, "w2": W2,
        "bqh": head_bias(bq_s), "bkh": head_bias(bk), "bvh": head_bias(bv),
        "b1p": b1p.reshape(4, P).copy(), "b2r": b2.reshape(1, D),
        "gbc": np.broadcast_to(ln_g, (P, D)).copy(),
        "bbc": np.broadcast_to(ln_b, (P, D)).copy(),
    }
    in_maps = []
    for b in range(B):
        masks = np.stack([(st[b] == p) for p in range(NS)]).astype(np.float32)
        xm = np.ascontiguousarray(
            (x[b][None, :, :] * masks[:, :, None]).transpose(0, 2, 1))
        in_maps.append({"xm": xm, **common})
    return in_maps


def kernel(**inputs):
    from concourse.bass_utils import run_bass_kernel_spmd
    if "nc" not in _CACHE:
        _CACHE["nc"] = _build()
    nc = _CACHE["nc"]
    in_maps = _host_prep(inputs)
    res = run_bass_kernel_spmd(nc, in_maps, list(range(NCORES)))
    out = np.stack([res.results[b]["out"] for b in range(B)])
    aw = np.stack([np.ascontiguousarray(res.results[b]["aw"].T)
                   for b in range(B)])
    return out, aw
